# revision 4
# baseline (speedup 1.0000x reference)
"""Single-head causal attention (B=4, T=2048, C=2048, H=128) on 8 TRN2 cores.

Partial-attention sharding, no inter-core communication: 2 cores per batch.
Core (2b + par) owns the 8 key tiles {128*(2m+par)} of batch b and computes
  - K^T, V^T for its 1024 own key positions only,
  - Q^T for ALL 2048 query positions of the batch,
  - partial attention:   ot = sum_{own k} exp(s) * v,  den = sum_{own k} exp(s)
over every query. The host adds the two cores' partials per batch and divides
(softmax sums commute; the max-shift is skipped since |s| < ~6, as in the
reference-matched baseline).

Per-core x.T is column-permuted to [own tiles | sib tiles]; with that order
the kernel is SPMD-identical:
  - K/V project from cols [0, 1024), Q from all cols,
  - attention key tile j covers query cols [128j, 1024) of BOTH halves:
    own half diag block gets the triangular mask, sib half diag block gets a
    per-core scalar (par=0 -> 1.0 since sib queries 2m+1 > own keys 2m;
    par=1 -> 0.0 since sib queries 2m < own keys 2m+1).

Pipeline per core (bf16 matmuls, fp32 PSUM):
  phase 1 (DMA-paced, two DMA rings): fused K/V/Q accumulation per x chunk,
    all 8 PSUM banks live (K:2, V:2, Q:4).
  copies to SBUF, 8 V-tile PE transposes.
  attention per half (own cols then sib cols), j = key tile 0..7:
    S[j] = K_j^T @ Q[:, base+128j : base+1024]   (<=2 single-bank matmuls)
    A = exp(SCALE * S) on ACT; mask 128-col diag block (tri | odd scalar)
    O += V_j @ A ; A_sum += A on DVE (vector)    <- den off the PE
    den = ones^T @ A_sum_bf16 (one 2-matmul pass per half)
  outputs ot [128, 2048] f32, den [1, 2048] f32 (partial, host combines).
"""

import numpy as np
import ml_dtypes

B, T, C, H = 4, 2048, 2048, 128
P = 128                 # tile edge
NCT = C // P            # 16 contraction c-tiles
NKT = 8                 # own key tiles per core
NQ = 2048               # query cols processed per core (own 1024 | sib 1024)
N_CORES = 8
SCALE = float(H) ** -0.5
BF16 = ml_dtypes.bfloat16

# x chunk widths in c-tiles (sum = 16); small first chunks start PE earlier
CHUNKS = [1, 1, 2, 2, 2, 2, 2, 2, 2]

_cache = {}


def _build():
    import concourse.bass as bass
    import concourse.mybir as mybir
    import concourse.tile as tile
    from concourse import bacc
    from concourse.masks import make_identity, make_upper_triangular

    dt = mybir.dt
    nc = bacc.Bacc(
        "TRN2",
        target_bir_lowering=False,
        debug=False,
        enable_asserts=False,
        num_devices=N_CORES,
    )

    xkvT = nc.dram_tensor("xkvT", [C, T], dt.bfloat16, kind="ExternalInput").ap()
    wq_d = nc.dram_tensor("wq", [P, NCT, H], dt.bfloat16, kind="ExternalInput").ap()
    wk_d = nc.dram_tensor("wk", [P, NCT, H], dt.bfloat16, kind="ExternalInput").ap()
    wv_d = nc.dram_tensor("wv", [P, NCT, H], dt.bfloat16, kind="ExternalInput").ap()
    # sib-half diag block allowed: 1.0 on par=0 cores, 0.0 on par=1 cores
    odd_d = nc.dram_tensor("odd", [P, 1], dt.float32, kind="ExternalInput").ap()
    ot_d = nc.dram_tensor("ot", [H, NQ], dt.float32, kind="ExternalOutput").ap()
    den_d = nc.dram_tensor("den", [1, NQ], dt.float32, kind="ExternalOutput").ap()

    with tile.TileContext(nc) as tc:
        with (
            tc.tile_pool(name="persist", bufs=1) as persist,
            tc.tile_pool(name="ephem", bufs=4) as ephem,
            tc.tile_pool(name="outp", bufs=2) as outp,
            tc.tile_pool(name="psum", bufs=1, space="PSUM") as psum,
        ):
            def bank(b, shape=(P, 512), dtype=dt.float32, name="pb"):
                return psum.tile(list(shape), dtype, tag=f"bank{b}", name=f"{name}{b}")

            wq_sb = persist.tile([P, NCT, H], dt.bfloat16)
            wk_sb = persist.tile([P, NCT, H], dt.bfloat16)
            wv_sb = persist.tile([P, NCT, H], dt.bfloat16)
            odd_sb = persist.tile([P, 1], dt.float32)
            xg_sb = [
                persist.tile([P, w, T], dt.bfloat16, name=f"xg{g}")
                for g, w in enumerate(CHUNKS)
            ]
            k_sb = persist.tile([P, P * NKT], dt.bfloat16)   # K^T own [h, 1024]
            vt_sb = persist.tile([P, P * NKT], dt.bfloat16)  # V^T own [h, 1024]
            v_sb = persist.tile([P, NKT, H], dt.bfloat16)    # V tiles [k, h]
            q_sb = persist.tile([P, NQ], dt.bfloat16)        # Q^T [h, 2048]
            asum = [
                persist.tile([P, 1024], dt.float32, name=f"asum{h}") for h in (0, 1)
            ]
            abf = [
                persist.tile([P, 1024], dt.bfloat16, name=f"abf{h}") for h in (0, 1)
            ]
            ident = persist.tile([P, P], dt.bfloat16)
            tri = persist.tile([P, P], dt.bfloat16)          # 1 where k <= q
            ones_sb = persist.tile([P, 1], dt.bfloat16)

            nc.sync.dma_start(out=wk_sb[:], in_=wk_d[:])
            nc.sync.dma_start(out=wv_sb[:], in_=wv_d[:])
            nc.sync.dma_start(out=wq_sb[:], in_=wq_d[:])
            nc.sync.dma_start(out=odd_sb[:], in_=odd_d[:])
            make_identity(nc, ident[:])
            make_upper_triangular(nc, tri[:], val=1.0, diag=True)
            nc.vector.memset(ones_sb[:], 1.0)
            # preload the ACT exp table off the attention critical path
            warm_sb = persist.tile([P, 1], dt.float32)
            nc.scalar.activation(
                warm_sb[:], ones_sb[:], mybir.ActivationFunctionType.Exp
            )

            # ---- phase 1: pipelined x load + fused K/V/Q accumulation ----
            # banks 0-1: K; banks 2-3: V; banks 4-7: Q
            ps_k = [bank(n, name="psk") for n in (0, 1)]
            ps_v = [bank(n, name="psv") for n in (2, 3)]
            ps_q = [bank(n, name="psq") for n in (4, 5, 6, 7)]
            c_lo = 0
            for g, w in enumerate(CHUNKS):
                eng = nc.scalar
                eng.dma_start(
                    out=xg_sb[g][:],
                    in_=xkvT[P * c_lo:P * (c_lo + w), :].rearrange(
                        "(j p) t -> p j t", p=P
                    ),
                )
                for jj in range(w):
                    j = c_lo + jj
                    st, sp = j == 0, j == NCT - 1
                    for n in range(2):
                        nc.tensor.matmul(
                            ps_k[n][:],
                            lhsT=wk_sb[:, j, :],
                            rhs=xg_sb[g][:, jj, 512 * n:512 * (n + 1)],
                            start=st, stop=sp,
                        )
                    for n in range(2):
                        nc.tensor.matmul(
                            ps_v[n][:],
                            lhsT=wv_sb[:, j, :],
                            rhs=xg_sb[g][:, jj, 512 * n:512 * (n + 1)],
                            start=st, stop=sp,
                        )
                    for n in range(4):
                        nc.tensor.matmul(
                            ps_q[n][:],
                            lhsT=wq_sb[:, j, :],
                            rhs=xg_sb[g][:, jj, 512 * n:512 * (n + 1)],
                            start=st, stop=sp,
                        )
                c_lo += w

            # copies ordered so attention can start early:
            # k first chunk, q own half, then v (for transposes), then rest
            nc.vector.tensor_copy(k_sb[:, 0:512], ps_k[0][:])
            nc.vector.tensor_copy(q_sb[:, 0:512], ps_q[0][:])
            nc.vector.tensor_copy(q_sb[:, 512:1024], ps_q[1][:])
            nc.vector.tensor_copy(vt_sb[:, 0:512], ps_v[0][:])
            nc.vector.tensor_copy(vt_sb[:, 512:1024], ps_v[1][:])
            nc.vector.tensor_copy(k_sb[:, 512:1024], ps_k[1][:])
            nc.vector.tensor_copy(q_sb[:, 1024:1536], ps_q[2][:])
            nc.vector.tensor_copy(q_sb[:, 1536:2048], ps_q[3][:])

            # ---- V tiles via PE transpose (banks 0-1 ping-pong) ----
            for kt in range(NKT):
                ps_t = bank(kt % 2, shape=(P, P), dtype=dt.bfloat16, name="pst")
                nc.tensor.transpose(
                    ps_t[:], vt_sb[:, kt * P:(kt + 1) * P], ident[:]
                )
                nc.vector.tensor_copy(v_sb[:, kt, :], ps_t[:])

            # ---- attention: two query halves (own, sib) ----
            # banks 2-3: O accum; S pairs rotate over (4,5), (6,7), (0,1)
            s_pairs = [(4, 5), (6, 7), (0, 1)]

            def attention_half(half):
                base = 1024 * half
                ps_o = [bank(n, name=f"pso{half}_") for n in (2, 3)]
                for j in range(NKT):
                    c0 = P * j
                    pa, pb = s_pairs[j % 3]
                    a_sb = ephem.tile([P, 1024], dt.bfloat16, name="a_sb")
                    # S sub-tiles: cols [c0, 512) on bank pa, [512, 1024) on pb
                    if c0 < 512:
                        ps_sa = bank(pa, name="pss")
                        nc.tensor.matmul(
                            ps_sa[:, c0:512],
                            lhsT=k_sb[:, c0:c0 + P],
                            rhs=q_sb[:, base + c0:base + 512],
                            start=True, stop=True,
                        )
                        nc.scalar.activation(
                            a_sb[:, c0:512], ps_sa[:, c0:512],
                            mybir.ActivationFunctionType.Exp,
                            scale=SCALE,
                        )
                        lo2 = 512
                    else:
                        lo2 = c0
                    ps_sb = bank(pb, name="pss")
                    nc.tensor.matmul(
                        ps_sb[:, lo2 - 512:512],
                        lhsT=k_sb[:, c0:c0 + P],
                        rhs=q_sb[:, base + lo2:base + 1024],
                        start=True, stop=True,
                    )
                    nc.scalar.activation(
                        a_sb[:, lo2:1024], ps_sb[:, lo2 - 512:512],
                        mybir.ActivationFunctionType.Exp,
                        scale=SCALE,
                    )
                    # diag block mask
                    if half == 0:
                        nc.vector.tensor_mul(
                            a_sb[:, c0:c0 + P], a_sb[:, c0:c0 + P], tri[:]
                        )
                    else:
                        nc.vector.tensor_scalar_mul(
                            a_sb[:, c0:c0 + P], a_sb[:, c0:c0 + P], odd_sb[:]
                        )
                    # O accumulation (bank 2: cols 0:512 j<=3; bank 3: all j)
                    if c0 < 512:
                        nc.tensor.matmul(
                            ps_o[0][:, c0:512],
                            lhsT=v_sb[:, j, :],
                            rhs=a_sb[:, c0:512],
                            start=j == 0, stop=j == 3,
                        )
                    nc.tensor.matmul(
                        ps_o[1][:, lo2 - 512:512],
                        lhsT=v_sb[:, j, :],
                        rhs=a_sb[:, lo2:1024],
                        start=j == 0, stop=j == NKT - 1,
                    )
                    # den accumulation on DVE
                    if j == 0:
                        nc.vector.tensor_copy(asum[half][:, 0:1024], a_sb[:])
                    else:
                        nc.vector.tensor_add(
                            asum[half][:, c0:1024],
                            asum[half][:, c0:1024],
                            a_sb[:, c0:1024],
                        )

                # den = ones^T @ bf16(A_sum)
                nc.vector.tensor_copy(abf[half][:], asum[half][:])
                den_sb = outp.tile([1, 1024], dt.float32, name="den_sb")
                for n in range(2):
                    ps_d = bank(s_pairs[0][n], shape=(1, 512), name="psd")
                    nc.tensor.matmul(
                        ps_d[:],
                        lhsT=ones_sb[:],
                        rhs=abf[half][:, 512 * n:512 * (n + 1)],
                        start=True, stop=True,
                    )
                    nc.vector.tensor_copy(den_sb[:, 512 * n:512 * (n + 1)], ps_d[:])
                nc.sync.dma_start(out=den_d[:, base:base + 1024], in_=den_sb[:])

                ot_sb = outp.tile([P, 1024], dt.float32, name="ot_sb")
                for n in range(2):
                    nc.vector.tensor_copy(
                        ot_sb[:, 512 * n:512 * (n + 1)], ps_o[n][:]
                    )
                nc.sync.dma_start(out=ot_d[:, base:base + 1024], in_=ot_sb[:])

            attention_half(0)
            attention_half(1)

    nc.compile()
    return nc


def _core_perm(core):
    par = core % 2
    own = [2 * m + par for m in range(NKT)]
    sib = [2 * m + 1 - par for m in range(NKT)]
    return own + sib


def _prep_inputs(x, Wq, Wk, Wv):
    """Build the 8 per-core input maps."""
    def wshape(w):
        # [C, H] -> [128, NCT, H]: w_r[p, j, h] = w[j*128 + p, h]
        return np.ascontiguousarray(
            w.astype(BF16).reshape(NCT, P, H).transpose(1, 0, 2)
        )

    wq_b, wk_b, wv_b = wshape(Wq), wshape(Wk), wshape(Wv)
    x_bf = x.astype(BF16)

    in_maps = []
    for core in range(N_CORES):
        b, par = core // 2, core % 2
        cols = np.concatenate(
            [np.arange(P * t, P * t + P) for t in _core_perm(core)]
        )
        xT = np.ascontiguousarray(x_bf[b].T[:, cols])
        odd = np.full((P, 1), 1.0 - par, np.float32)
        in_maps.append({
            "xkvT": xT,
            "wq": wq_b, "wk": wk_b, "wv": wv_b,
            "odd": np.ascontiguousarray(odd),
        })
    return in_maps


def _assemble(results):
    num = np.zeros((B, T, H), np.float32)
    den = np.zeros((B, T, 1), np.float32)
    for core in range(N_CORES):
        b = core // 2
        r = results[core]
        oT = r["ot"].T          # [2048, H]
        dT = r["den"].T         # [2048, 1]
        for i, g in enumerate(_core_perm(core)):
            num[b, P * g:P * (g + 1)] += oT[P * i:P * (i + 1)]
            den[b, P * g:P * (g + 1)] += dT[P * i:P * (i + 1)]
    return num / den


def _run(inputs, trace=False, **spmd_kwargs):
    from concourse.bass_utils import run_bass_kernel_spmd

    if "nc" not in _cache:
        _cache["nc"] = _build()
    nc = _cache["nc"]
    in_maps = _prep_inputs(
        np.asarray(inputs["x"], np.float32),
        np.asarray(inputs["Wq"], np.float32),
        np.asarray(inputs["Wk"], np.float32),
        np.asarray(inputs["Wv"], np.float32),
    )
    res = run_bass_kernel_spmd(
        nc, in_maps, list(range(N_CORES)), trace=trace, **spmd_kwargs
    )
    return _assemble(res.results), res


def kernel(x, Wq, Wk, Wv):
    out, _ = _run({"x": x, "Wq": Wq, "Wk": Wk, "Wv": Wv})
    return out
